# revision 1
# baseline (speedup 1.0000x reference)
"""Trainium2 Bass kernel for DihedralToCartesian (NeRF-style dihedral->xyz chain).

Full-input contract: kernel(angles[65536,252], prev_three[65536,3,3]) -> [65536,126,3].
Internally: batch is sharded 8 ways (8192 rows/core, pure data parallelism).

Math restructuring (validated vs the JAX reference, full batch, rel err ~3e-5):
the per-atom step
    bc = norm(b-c); n = norm((b-a) x bc); m1 = n x bc
    d  = c + r0*bc + r1*m1 + r2*n
is an affine chain over a frame F = [f1, f2, f3] = [bc, m1, n]:
    h    = cB*f2 - sB*f3          (cB,sB = eps-damped cos/sin of dihedral)
    p   += bond*cosA*f1 + bond*sinA*h
    f1'  = (-cosA*f1 - sinA*h) * invw
    f2'  = (sinA*g2*f1 - cosA*h) * invw*invg
    f3'  = (sB*f2 + cB*f3) * invg
where g2 = cB^2+sB^2 (slightly < 1 when sin^2+cos^2 is tiny, because the
reference adds 1e-8 inside its normalize), invg = rsqrt(g2) and
invw = rsqrt(cos^2 A + sin^2 A * g2).  These per-atom normalizers reproduce
exactly the frame tilt the reference gets from renormalizing eps-damped
vectors; without them, rare tiny-dihedral rows diverge to ~1e-2.

Per-core layout: batch row beta = 64*p + j (p = SBUF partition, j in [0,64)).
Recurrence ops are [128, 3, 64] fp32 (comp-planar).  All rsqrts are done as
exp(-0.5*ln(x)) on ScalarE (the Rsqrt table is banned as inaccurate).
Dihedral precompute is chunked by atom range and interleaved with the step
loop so it hides behind the recurrence.
"""

import os
import sys

import numpy as np

for _p in ("/opt/trn_rl_repo", os.path.expanduser("~/.axon_site/_ro/trn_rl_repo")):
    if os.path.isdir(_p) and _p not in sys.path:
        sys.path.insert(0, _p)

import concourse.bass as bass
import concourse.bacc as bacc
import concourse.mybir as mybir
import concourse.tile as tile
from concourse.bass_utils import run_bass_kernel_spmd

F32 = mybir.dt.float32
AOP = mybir.AluOpType
AF = mybir.ActivationFunctionType

N_CORES = 8
B_FULL = 65536
BS = B_FULL // N_CORES  # 8192 rows per core
N = 126                 # atoms
P = 128                 # partitions
J = BS // P             # 64 batch columns per partition
BLK = 18                # atoms per output staging block
CH_A = 6                # atoms per precompute chunk

_ALPHA = np.array([2.028, 2.124, 1.941], np.float32)
_BOND = np.array([1.329, 1.458, 1.523], np.float32)
_CA = np.cos(_ALPHA)
_SA = np.sin(_ALPHA)
_C2A = (_CA * _CA).astype(np.float32)
_S2A = (np.float32(1.0) - _C2A).astype(np.float32)
_BCA = _BOND * _CA
_BSA = _BOND * _SA


def _emit(nc: bass.Bass):
    angles = nc.dram_tensor("angles", [BS, 2 * N], F32, kind="ExternalInput").ap()
    prev = nc.dram_tensor("prev_three", [BS, 3, 3], F32, kind="ExternalInput").ap()
    out = nc.dram_tensor("out", [BS, N, 3], F32, kind="ExternalOutput").ap()

    ang_r = angles.rearrange("(p j) c -> p j c", p=P)          # [128, 64, 252]
    prev_r = prev.rearrange("(p j) r c -> p j (r c)", p=P)     # [128, 64, 9]
    out_r = out.rearrange("(p j) a c -> p j (a c)", p=P)       # [128, 64, 378]

    with tile.TileContext(nc) as tc:
        with (
            tc.tile_pool(name="planes", bufs=1) as planes,
            tc.tile_pool(name="stag", bufs=2) as stagp,
            tc.tile_pool(name="chunk", bufs=1) as chunk,
            tc.tile_pool(name="state", bufs=2) as state,
            tc.tile_pool(name="scratch", bufs=2) as scratch,
        ):
            # persistent planes, f = 126*j + a
            rawS = planes.tile([P, J * N], F32, tag="rawS")  # raw sin -> invg
            rawC = planes.tile([P, J * N], F32, tag="rawC")  # raw cos -> invw
            cdP = planes.tile([P, J * N], F32, tag="cdP")    # damped cos(theta)
            sdP = planes.tile([P, J * N], F32, tag="sdP")    # damped sin(theta)
            pv = planes.tile([P, J * 9], F32, tag="pv")
            c2aT = planes.tile([P, N], F32, tag="c2aT")      # cos^2(alpha) pattern
            s2aT = planes.tile([P, N], F32, tag="s2aT")      # sin^2(alpha) pattern

            nc.sync.dma_start(
                out=rawS[:].rearrange("p (j a) -> p j a", a=N), in_=ang_r[:, :, 0:N]
            )
            nc.sync.dma_start(
                out=rawC[:].rearrange("p (j a) -> p j a", a=N),
                in_=ang_r[:, :, N : 2 * N],
            )
            nc.sync.dma_start(
                out=pv[:].rearrange("p (j x) -> p j x", x=9), in_=prev_r
            )
            for k in range(3):
                v3 = c2aT[:].rearrange("p (a k) -> p a k", k=3)[:, :, k]
                nc.vector.memset(v3, float(_C2A[k]))
                v3 = s2aT[:].rearrange("p (a k) -> p a k", k=3)[:, :, k]
                nc.vector.memset(v3, float(_S2A[k]))

            # atom-major views [128, a, j] / chunk views [128, j, a]
            def aview(t):
                return t[:].rearrange("p (j a) -> p a j", a=N)

            def jview(t):
                return t[:].rearrange("p (j a) -> p j a", a=N)

            # ---- initial frame from prev_three -------------------------------
            pv_r = pv[:].rearrange("p (j x) -> p x j", x=9)      # [128, 9, 64]
            a_ap = pv_r[:, 0:3, :]
            b_ap = pv_r[:, 3:6, :]
            c_ap = pv_r[:, 6:9, :]

            def cross(dst, x, y, eps):
                for c in range(3):
                    c1, c2 = (c + 1) % 3, (c + 2) % 3
                    m = scratch.tile([P, 1, J], F32, tag="cr_m")
                    qt = scratch.tile([P, 1, J], F32, tag="cr_q")
                    nc.vector.tensor_mul(m[:], x[:, c1 : c1 + 1, :], y[:, c2 : c2 + 1, :])
                    nc.vector.tensor_mul(qt[:], x[:, c2 : c2 + 1, :], y[:, c1 : c1 + 1, :])
                    nc.vector.scalar_tensor_tensor(
                        dst[:, c : c + 1, :], m[:], eps, qt[:], AOP.add, AOP.subtract
                    )

            def rsqrt3(dst, src3):
                sq = scratch.tile([P, 3, J], F32, tag="in_sq")
                nc.scalar.square(sq[:], src3[:])
                s1 = scratch.tile([P, J], F32, tag="in_s1")
                nc.vector.tensor_add(s1[:], sq[:, 0, :], sq[:, 1, :])
                s2_ = scratch.tile([P, J], F32, tag="in_s2")
                nc.vector.tensor_add(s2_[:], s1[:], sq[:, 2, :])
                lgi = scratch.tile([P, J], F32, tag="in_lg")
                nc.scalar.activation(lgi[:], s2_[:], AF.Ln)
                nc.scalar.activation(dst[:], lgi[:], AF.Exp, 0.0, -0.5)

            vv = scratch.tile([P, 3, J], F32, tag="in_v")
            nc.vector.scalar_tensor_tensor(
                vv[:], b_ap, 1e-8, c_ap, AOP.add, AOP.subtract
            )
            rv1 = scratch.tile([P, J], F32, tag="in_rv")
            rsqrt3(rv1, vv)
            f1 = state.tile([P, 3, J], F32, tag="f1")
            nc.vector.tensor_mul(
                f1[:], vv[:], rv1[:].unsqueeze(1).broadcast_to([P, 3, J])
            )
            uu = scratch.tile([P, 3, J], F32, tag="in_u")
            nc.vector.tensor_sub(uu[:], b_ap, a_ap)
            ww = scratch.tile([P, 3, J], F32, tag="in_w")
            cross(ww, uu, f1, 1e-8)
            rw = scratch.tile([P, J], F32, tag="in_rw")
            rsqrt3(rw, ww)
            f3 = state.tile([P, 3, J], F32, tag="f3")
            nc.vector.tensor_mul(
                f3[:], ww[:], rw[:].unsqueeze(1).broadcast_to([P, 3, J])
            )
            f2 = state.tile([P, 3, J], F32, tag="f2")
            cross(f2, f3, f1, 0.0)

            # ---- fused: precompute chunks interleaved with the chain ---------
            p_prev_ap = c_ap
            stag_tiles = [None, None]

            def emit_chunk(k):
                asl = slice(CH_A * k, CH_A * (k + 1))
                rS = jview(rawS)[:, :, asl]
                rC = jview(rawC)[:, :, asl]
                cD = jview(cdP)[:, :, asl]
                sD = jview(sdP)[:, :, asl]
                SH = [P, J, CH_A]
                s2 = chunk.tile(SH, F32, tag="s2")
                nc.scalar.square(s2[:], rS)
                c2 = chunk.tile(SH, F32, tag="c2")
                nc.scalar.square(c2[:], rC)
                ss = chunk.tile(SH, F32, tag="ss")
                nc.vector.scalar_tensor_tensor(
                    ss[:], s2[:], 1e-8, c2[:], AOP.add, AOP.add
                )
                lg = chunk.tile(SH, F32, tag="lg")
                nc.scalar.activation(lg[:], ss[:], AF.Ln)
                rv = chunk.tile(SH, F32, tag="rv")
                nc.scalar.activation(rv[:], lg[:], AF.Exp, 0.0, -0.5)
                nc.vector.tensor_mul(cD, rC, rv[:])
                nc.vector.tensor_mul(sD, rS, rv[:])
                gc = chunk.tile(SH, F32, tag="s2", name=f"gc{k}")
                nc.scalar.square(gc[:], cD)
                gs = chunk.tile(SH, F32, tag="c2", name=f"gs{k}")
                nc.scalar.square(gs[:], sD)
                gg = chunk.tile(SH, F32, tag="gg")
                nc.gpsimd.tensor_add(gg[:], gc[:], gs[:])
                lg2 = chunk.tile(SH, F32, tag="lg", name=f"lg2_{k}")
                nc.scalar.activation(lg2[:], gg[:], AF.Ln)
                nc.scalar.activation(rS, lg2[:], AF.Exp, 0.0, -0.5)  # invg -> rawS
                c2a_b = (
                    c2aT[:, asl].unsqueeze(1).broadcast_to([P, J, CH_A])
                )
                s2a_b = (
                    s2aT[:, asl].unsqueeze(1).broadcast_to([P, J, CH_A])
                )
                mw = chunk.tile(SH, F32, tag="mw")
                nc.vector.tensor_mul(mw[:], gg[:], s2a_b)
                w2 = chunk.tile(SH, F32, tag="w2")
                nc.vector.tensor_add(w2[:], mw[:], c2a_b)
                lg3 = chunk.tile(SH, F32, tag="lg", name=f"lg3_{k}")
                nc.scalar.activation(lg3[:], w2[:], AF.Ln)
                nc.scalar.activation(rC, lg3[:], AF.Exp, 0.0, -0.5)  # invw -> rawC

            cdA, sdA, igA, iwA = aview(cdP), aview(sdP), aview(rawS), aview(rawC)

            for i in range(N):
                if i % CH_A == 0:
                    emit_chunk(i // CH_A)
                k3 = i % 3
                ca, sa = float(_CA[k3]), float(_SA[k3])
                bca, bsa = float(_BCA[k3]), float(_BSA[k3])
                blk, al = i // BLK, i % BLK
                last = i == N - 1
                if al == 0:
                    stag_tiles[blk % 2] = stagp.tile(
                        [P, J * 3 * BLK], F32, tag="stag", name=f"stag{blk}"
                    )
                stag = stag_tiles[blk % 2]
                stag_r = stag[:].rearrange("p (j x) -> p x j", x=3 * BLK)

                cb1 = cdA[:, i : i + 1, :]
                sb1 = sdA[:, i : i + 1, :]
                ig1 = igA[:, i : i + 1, :]
                iw1 = iwA[:, i : i + 1, :]
                cb = cb1.broadcast_to([P, 3, J])
                sb = sb1.broadcast_to([P, 3, J])
                ig_b = ig1.broadcast_to([P, 3, J])
                iw_b = iw1.broadcast_to([P, 3, J])

                if not last:
                    # per-atom g^2 and invw*invg (small [128,1,64] ops)
                    sqc = scratch.tile([P, 1, J], F32, tag="sqc")
                    nc.scalar.square(sqc[:], cb1)
                    sqs = scratch.tile([P, 1, J], F32, tag="sqs")
                    nc.scalar.square(sqs[:], sb1)
                    ggs = scratch.tile([P, 1, J], F32, tag="ggs")
                    nc.vector.tensor_add(ggs[:], sqc[:], sqs[:])
                    iwg = scratch.tile([P, 1, J], F32, tag="iwg")
                    nc.vector.tensor_mul(iwg[:], iw1, ig1)
                    # early ACT/DVE work off the critical h-chain
                    fc = scratch.tile([P, 3, J], F32, tag="fc")
                    nc.scalar.mul(fc[:], f1[:], ca)
                    t9 = scratch.tile([P, 3, J], F32, tag="t9")
                    nc.vector.tensor_mul(
                        t9[:], f1[:], ggs[:].broadcast_to([P, 3, J])
                    )
                    t9s = scratch.tile([P, 3, J], F32, tag="t9s")
                    nc.scalar.mul(t9s[:], t9[:], sa)

                tmp = scratch.tile([P, 3, J], F32, tag="tmp")
                nc.vector.scalar_tensor_tensor(
                    tmp[:], f1[:], bca, p_prev_ap, AOP.mult, AOP.add
                )
                t1 = scratch.tile([P, 3, J], F32, tag="t1")
                nc.vector.tensor_mul(t1[:], f2[:], cb)
                t2 = scratch.tile([P, 3, J], F32, tag="t2")
                nc.vector.tensor_mul(t2[:], f3[:], sb)
                h = scratch.tile([P, 3, J], F32, tag="h")
                nc.vector.tensor_sub(h[:], t1[:], t2[:])

                pn_ap = stag_r[:, 3 * al : 3 * al + 3, :]
                nc.vector.scalar_tensor_tensor(
                    pn_ap, h[:], bsa, tmp[:], AOP.mult, AOP.add
                )

                if not last:
                    f1p = scratch.tile([P, 3, J], F32, tag="f1p")
                    nc.vector.scalar_tensor_tensor(
                        f1p[:], h[:], -sa, fc[:], AOP.mult, AOP.subtract
                    )
                    f1n = state.tile([P, 3, J], F32, tag="f1")
                    nc.vector.tensor_mul(f1n[:], f1p[:], iw_b)
                    f2p = scratch.tile([P, 3, J], F32, tag="f2p")
                    nc.vector.scalar_tensor_tensor(
                        f2p[:], h[:], -ca, t9s[:], AOP.mult, AOP.add
                    )
                    f2n = state.tile([P, 3, J], F32, tag="f2")
                    nc.vector.tensor_mul(
                        f2n[:], f2p[:], iwg[:].broadcast_to([P, 3, J])
                    )
                    t3 = scratch.tile([P, 3, J], F32, tag="t3")
                    nc.gpsimd.tensor_mul(t3[:], f2[:], sb)
                    t4 = scratch.tile([P, 3, J], F32, tag="t4")
                    nc.gpsimd.tensor_mul(t4[:], f3[:], cb)
                    f3s = scratch.tile([P, 3, J], F32, tag="f3s")
                    nc.gpsimd.tensor_add(f3s[:], t3[:], t4[:])
                    f3n = state.tile([P, 3, J], F32, tag="f3")
                    nc.gpsimd.tensor_mul(f3n[:], f3s[:], ig_b)
                    f1, f2, f3 = f1n, f2n, f3n
                p_prev_ap = pn_ap

                if al == BLK - 1:
                    nc.sync.dma_start(
                        out=out_r[:, :, 3 * BLK * blk : 3 * BLK * (blk + 1)],
                        in_=stag[:].rearrange("p (j x) -> p j x", x=3 * BLK),
                    )
    return nc


_NC_CACHE: dict = {}


def _get_nc():
    if "nc" not in _NC_CACHE:
        nc = bacc.Bacc("TRN2", target_bir_lowering=False, debug=False)
        _emit(nc)
        nc.compile()
        _NC_CACHE["nc"] = nc
    return _NC_CACHE["nc"]


def run_sharded(angles: np.ndarray, prev_three: np.ndarray, **kw):
    """Shard inputs over 8 cores, run, return BassKernelResults."""
    angles = np.ascontiguousarray(angles, np.float32)
    prev_three = np.ascontiguousarray(prev_three, np.float32)
    assert angles.shape == (B_FULL, 2 * N) and prev_three.shape == (B_FULL, 3, 3)
    in_maps = [
        {
            "angles": angles[i * BS : (i + 1) * BS],
            "prev_three": prev_three[i * BS : (i + 1) * BS],
        }
        for i in range(N_CORES)
    ]
    return run_bass_kernel_spmd(_get_nc(), in_maps, core_ids=list(range(N_CORES)), **kw)


def kernel(angles: np.ndarray, prev_three: np.ndarray) -> np.ndarray:
    res = run_sharded(angles, prev_three)
    return np.concatenate([r["out"] for r in res.results], axis=0)



# revision 5
# speedup vs baseline: 1.3816x; 1.3816x over previous
"""Trainium2 Bass kernel for DihedralToCartesian (NeRF-style dihedral->xyz chain).

Full-input contract: kernel(angles[65536,252], prev_three[65536,3,3]) -> [65536,126,3].
Batch sharded 8 ways (8192 rows/core, pure data parallelism).

Math (validated vs the JAX reference, rel err ~3e-3, tolerance 2e-2):
the reference's per-atom frame update with eps-damped normalizers reduces to
    h'   = cd*f2 - sd*f3          (cd,sd = damped cos/sin(theta) * invg, |h'|=1)
    f1'  = -cosA*f1 - sinA*h'
    f2'  =  sinA*f1 - cosA*h'
    f3'  =  sd*f2 + cd*f3
    v    = bond*cosA*f1 + bond*sinA*h'
    d_i  = d_{i-1} + v_i
where invg is folded into cd/sd at precompute (keeps the frame norm-preserving)
and the remaining invw/g normalizers are ~1+O(1e-8*rn2), dropped (checked on the
full real batch: max rel err 3.1e-3).

Per-core layout: batch row = 64*p + j (p = SBUF partition, j in [0,64)).
The A-side coefficients are compile-time constants -> the chain is 5 DVE ops
per atom: one fused [128,12,64] mul (all four cd/sd products via a 5-dim
broadcast AP), one paired add producing (h', f3'), and three scalar_tensor_tensor
ops (f1', f2', v) whose f1-side products come from 3 ACT const-muls off the
critical path.  Positions are NOT in the chain: v is staged fp16 and cumsummed
by masked tensor_tensor_scan (fp32 internal state) per 42-atom chunk, then
DMA'd out as fp16 (host converts to fp32).
"""

import os
import sys

import numpy as np

for _p in ("/opt/trn_rl_repo", os.path.expanduser("~/.axon_site/_ro/trn_rl_repo")):
    if os.path.isdir(_p) and _p not in sys.path:
        sys.path.insert(0, _p)

import concourse.bass as bass
import concourse.bacc as bacc
import concourse.mybir as mybir
import concourse.tile as tile
from concourse.bass_utils import run_bass_kernel_spmd

F32 = mybir.dt.float32
F16 = mybir.dt.float16
AOP = mybir.AluOpType
AF = mybir.ActivationFunctionType

N_CORES = 8
B_FULL = 65536
BS = B_FULL // N_CORES  # 8192 rows per core
N = 126                 # atoms
P = 128                 # partitions
J = BS // P             # 64 batch columns per partition
CH = 14                 # atoms per precompute chunk (9 chunks)
CV = 42                 # atoms per v/scan/output chunk (3 chunks)

_ALPHA = np.array([2.028, 2.124, 1.941], np.float64)
_BOND = np.array([1.329, 1.458, 1.523], np.float64)
_CA = np.cos(_ALPHA)
_SA = np.sin(_ALPHA)
_BCA = _BOND * _CA
_BSA = _BOND * _SA


def _emit(nc: bass.Bass):
    angles = nc.dram_tensor("angles", [BS, 2 * N], F32, kind="ExternalInput").ap()
    prev = nc.dram_tensor("prev_three", [BS, 3, 3], F32, kind="ExternalInput").ap()
    out = nc.dram_tensor("out", [BS, N, 3], F16, kind="ExternalOutput").ap()

    ang_r = angles.rearrange("(p j) c -> p j c", p=P)          # [128, 64, 252]
    prev_r = prev.rearrange("(p j) r c -> p j (r c)", p=P)     # [128, 64, 9]
    out_r = out.rearrange("(p j) a c -> p j (a c)", p=P)       # [128, 64, 378]

    with tile.TileContext(nc) as tc:
        with (
            tc.tile_pool(name="planes", bufs=1) as planes,
            tc.tile_pool(name="coeff", bufs=2) as coeffp,
            tc.tile_pool(name="vout", bufs=2) as voutp,
            tc.tile_pool(name="state", bufs=1) as state,
            tc.tile_pool(name="pre", bufs=2) as pre,
            tc.tile_pool(name="scratch", bufs=2) as scratch,
        ):
            rawS = planes.tile([P, J * N], F32, tag="rawS")
            rawC = planes.tile([P, J * N], F32, tag="rawC")
            pv = planes.tile([P, J * 9], F32, tag="pv")
            mask = planes.tile([P, J * CV], F16, tag="mask")

            nc.sync.dma_start(
                out=rawS[:].rearrange("p (j a) -> p j a", a=N), in_=ang_r[:, :, 0:N]
            )
            nc.sync.dma_start(
                out=rawC[:].rearrange("p (j a) -> p j a", a=N),
                in_=ang_r[:, :, N : 2 * N],
            )
            nc.sync.dma_start(out=pv[:].rearrange("p (j x) -> p j x", x=9), in_=prev_r)

            mview = mask[:].rearrange("p (j a) -> p j a", a=CV)
            nc.vector.memset(mask[:], 1.0)
            nc.vector.memset(mview[:, :, 0:1], 0.0)
            epsb = planes.tile([P, 1], F32, tag="epsb")
            nc.vector.memset(epsb[:], 1e-8)

            # ---- chain state --------------------------------------------------
            # F: f2(0:3), h'(3:6), f3(6:9); f1 double-buffered in its own tiles.
            F = state.tile([P, 9, J], F32, tag="F")
            f1t = [
                state.tile([P, 3, J], F32, tag="f1a", name="f1a"),
                state.tile([P, 3, J], F32, tag="f1b", name="f1b"),
            ]
            U = state.tile([P, 12, J], F32, tag="U")
            fc = state.tile([P, 3, J], F32, tag="fc")
            fs = state.tile([P, 3, J], F32, tag="fs")
            fb = state.tile([P, 3, J], F32, tag="fb")
            vtmp = state.tile([P, 3, J], F16, tag="vtmp")

            # ---- initial frame from prev_three --------------------------------
            pv_r = pv[:].rearrange("p (j x) -> p x j", x=9)      # [128, 9, 64]
            a_ap = pv_r[:, 0:3, :]
            b_ap = pv_r[:, 3:6, :]
            c_ap = pv_r[:, 6:9, :]

            def cross(dst, x, y, eps):
                for c in range(3):
                    c1, c2 = (c + 1) % 3, (c + 2) % 3
                    m = scratch.tile([P, 1, J], F32, tag="cr_m")
                    qt = scratch.tile([P, 1, J], F32, tag="cr_q")
                    nc.vector.tensor_mul(m[:], x[:, c1 : c1 + 1, :], y[:, c2 : c2 + 1, :])
                    nc.vector.tensor_mul(qt[:], x[:, c2 : c2 + 1, :], y[:, c1 : c1 + 1, :])
                    nc.vector.scalar_tensor_tensor(
                        dst[:, c : c + 1, :], m[:], eps, qt[:], AOP.add, AOP.subtract
                    )

            def rsqrt3(dst, src3):
                sq = scratch.tile([P, 3, J], F32, tag="in_sq")
                nc.scalar.square(sq[:], src3[:])
                s1 = scratch.tile([P, J], F32, tag="in_s1")
                nc.vector.tensor_add(s1[:], sq[:, 0, :], sq[:, 1, :])
                s2_ = scratch.tile([P, J], F32, tag="in_s2")
                nc.vector.tensor_add(s2_[:], s1[:], sq[:, 2, :])
                lgi = scratch.tile([P, J], F32, tag="in_lg")
                nc.scalar.activation(lgi[:], s2_[:], AF.Ln)
                nc.scalar.activation(dst[:], lgi[:], AF.Exp, 0.0, -0.5)

            vv = scratch.tile([P, 3, J], F32, tag="in_v")
            nc.vector.scalar_tensor_tensor(
                vv[:], b_ap, 1e-8, c_ap, AOP.add, AOP.subtract
            )
            rv1 = scratch.tile([P, J], F32, tag="in_rv")
            rsqrt3(rv1, vv)
            f1_0 = f1t[0]
            nc.vector.tensor_mul(
                f1_0[:], vv[:], rv1[:].unsqueeze(1).broadcast_to([P, 3, J])
            )
            uu = scratch.tile([P, 3, J], F32, tag="in_u")
            nc.vector.tensor_sub(uu[:], b_ap, a_ap)
            ww = scratch.tile([P, 3, J], F32, tag="in_w")
            cross(ww, uu, f1_0, 1e-8)
            rw = scratch.tile([P, J], F32, tag="in_rw")
            rsqrt3(rw, ww)
            nc.vector.tensor_mul(
                F[:, 6:9, :], ww[:], rw[:].unsqueeze(1).broadcast_to([P, 3, J])
            )
            cross(F[:, 0:3, :], F[:, 6:9, :], f1_0, 0.0)

            # ---- coefficient precompute (windowed) ----------------------------
            c4_tiles = [None, None]

            def emit_chunk(q):
                asl = slice(CH * q, CH * (q + 1))
                rS = rawS[:].rearrange("p (j a) -> p j a", a=N)[:, :, asl]
                rC = rawC[:].rearrange("p (j a) -> p j a", a=N)[:, :, asl]
                SH = [P, J, CH]
                c4 = coeffp.tile([P, 4, J, CH], F16, tag="c4", name=f"c4_{q}")
                c4_tiles[q % 2] = c4
                s2 = pre.tile(SH, F32, tag="p_s2")
                nc.scalar.square(s2[:], rS)
                c2 = pre.tile(SH, F32, tag="p_c2")
                nc.scalar.square(c2[:], rC)
                nn = pre.tile(SH, F32, tag="p_n")
                nc.gpsimd.tensor_add(nn[:], s2[:], c2[:])
                lnn = pre.tile(SH, F32, tag="p_s2", name=f"lnn{q}")
                nc.scalar.activation(lnn[:], nn[:], AF.Ln, epsb[:], 1.0)
                rn = pre.tile(SH, F32, tag="p_c2", name=f"rn{q}")
                nc.scalar.activation(rn[:], lnn[:], AF.Exp, 0.0, -0.5)
                rn2 = pre.tile(SH, F32, tag="p_n", name=f"rn2{q}")
                nc.scalar.activation(rn2[:], lnn[:], AF.Exp, 0.0, -1.0)
                lng = pre.tile(SH, F32, tag="p_s2", name=f"lng{q}")
                nc.scalar.activation(lng[:], rn2[:], AF.Ln, 1.0, -1e-8)
                ig = pre.tile(SH, F32, tag="p_n", name=f"ig{q}")
                nc.scalar.activation(ig[:], lng[:], AF.Exp, 0.0, -0.5)
                r1 = pre.tile(SH, F32, tag="p_r1")
                nc.gpsimd.tensor_mul(r1[:], rn[:], ig[:])
                nr1 = pre.tile(SH, F32, tag="p_c2", name=f"nr1{q}")
                nc.scalar.mul(nr1[:], r1[:], -1.0)
                # C4 entries: 0=cd, 1=nsd, 2=sd, 3=cd  (layout [P,4,J,CH])
                rC_b = rC.unsqueeze(1).broadcast_to([P, 2, J, CH])
                r1_b = r1[:].unsqueeze(1).broadcast_to([P, 2, J, CH])
                nc.gpsimd.tensor_mul(c4[:, 0:4:3, :, :], rC_b, r1_b)
                nc.gpsimd.tensor_mul(c4[:, 2, :, :], rS, r1[:])
                nc.gpsimd.tensor_mul(c4[:, 1, :, :], rS, nr1[:])

            emit_chunk(0)

            # ---- the chain ----------------------------------------------------
            v_tiles = [None, None]
            o_tiles = [None, None]
            f23 = F[:, 0:9, :].rearrange("p (c k) j -> p c k j", c=3)[:, 0:3:2, :, :]
            f23b = f23.unsqueeze(1).broadcast_to([P, 2, 2, 3, J])
            uview = U[:].rearrange("p (r c k) j -> p r c k j", r=2, c=2)
            ug = U[:].rearrange("p (g k) j -> p g k j", g=4)
            hview = F[:, 3:6, :]

            for i in range(N):
                q, qa = divmod(i, CH)
                if qa == 0 and q + 1 < N // CH:
                    emit_chunk(q + 1)
                k, kv = divmod(i, CV)
                if kv == 0:
                    v_tiles[k % 2] = voutp.tile(
                        [P, J, CV, 3], F16, tag="vb", name=f"vb{k}"
                    )
                    o_tiles[k % 2] = voutp.tile(
                        [P, J, CV, 3], F16, tag="ob", name=f"ob{k}"
                    )
                vb = v_tiles[k % 2]
                p3 = i % 3
                ca, sa = float(_CA[p3]), float(_SA[p3])
                bca, bsa = float(_BCA[p3]), float(_BSA[p3])
                f1c = f1t[i % 2]
                f1n = f1t[(i + 1) % 2]
                c4 = c4_tiles[q % 2]

                # ACT const-muls off the critical path
                nc.scalar.mul(fc[:], f1c[:], -ca)
                nc.scalar.mul(fs[:], f1c[:], sa)
                nc.scalar.mul(fb[:], f1c[:], bca)

                # fused products: (cd*f2, nsd*f3 | sd*f2, cd*f3)
                c4v = (
                    c4[:, :, :, qa]
                    .rearrange("p (r c) j -> p r c j", r=2)
                    .unsqueeze(3)
                    .broadcast_to([P, 2, 2, 3, J])
                )
                nc.vector.tensor_mul(uview, c4v, f23b)
                # (h', f3') = pairwise sums
                nc.vector.tensor_add(
                    F[:, 3:9, :].rearrange("p (g k) j -> p g k j", g=2),
                    ug[:, 0:4:2, :, :],
                    ug[:, 1:4:2, :, :],
                )
                # f1' = -sa*h' + fc ; f2' = -ca*h' + fs ; v = bsa*h' + fb
                nc.vector.scalar_tensor_tensor(
                    f1n[:], hview, -sa, fc[:], AOP.mult, AOP.add
                )
                nc.vector.scalar_tensor_tensor(
                    F[:, 0:3, :], hview, -ca, fs[:], AOP.mult, AOP.add
                )
                if kv == 0:
                    vout = vtmp[:]
                else:
                    vout = vb[:, :, kv, :].rearrange("p j c -> p c j")
                nc.vector.scalar_tensor_tensor(
                    vout, hview, bsa, fb[:], AOP.mult, AOP.add
                )
                if kv == 0:
                    # fold carry (c0 for chunk 0, prev chunk's last d after)
                    if k == 0:
                        carry = c_ap.rearrange("p c j -> p j c")
                    else:
                        carry = o_tiles[(k + 1) % 2][:, :, CV - 1, :]
                    nc.gpsimd.tensor_add(
                        vb[:, :, 0, :], vtmp[:].rearrange("p c j -> p j c"), carry
                    )

                if kv == CV - 1:
                    ob = o_tiles[k % 2]
                    for c in range(3):
                        vvw = vb[:].rearrange("p j a c -> p c (j a)")[:, c, :]
                        oow = ob[:].rearrange("p j a c -> p c (j a)")[:, c, :]
                        nc.vector.tensor_tensor_scan(
                            oow, mask[:], vvw, 0.0, AOP.mult, AOP.add
                        )
                    nc.sync.dma_start(
                        out=out_r[:, :, 3 * CV * k : 3 * CV * (k + 1)],
                        in_=ob[:].rearrange("p j a c -> p j (a c)"),
                    )
    return nc


_NC_CACHE: dict = {}


def _get_nc():
    if "nc" not in _NC_CACHE:
        nc = bacc.Bacc("TRN2", target_bir_lowering=False, debug=False)
        _emit(nc)
        nc.compile()
        _NC_CACHE["nc"] = nc
    return _NC_CACHE["nc"]


def run_sharded(angles: np.ndarray, prev_three: np.ndarray, **kw):
    """Shard inputs over 8 cores, run, return BassKernelResults."""
    angles = np.ascontiguousarray(angles, np.float32)
    prev_three = np.ascontiguousarray(prev_three, np.float32)
    assert angles.shape == (B_FULL, 2 * N) and prev_three.shape == (B_FULL, 3, 3)
    in_maps = [
        {
            "angles": angles[i * BS : (i + 1) * BS],
            "prev_three": prev_three[i * BS : (i + 1) * BS],
        }
        for i in range(N_CORES)
    ]
    return run_bass_kernel_spmd(_get_nc(), in_maps, core_ids=list(range(N_CORES)), **kw)


def kernel(angles: np.ndarray, prev_three: np.ndarray) -> np.ndarray:
    res = run_sharded(angles, prev_three)
    return np.concatenate([r["out"] for r in res.results], axis=0).astype(np.float32)
